# revision 4
# baseline (speedup 1.0000x reference)
"""Low_Rank_linear Trainium2 kernel, v6.

Per 512-token block (data-parallel over 8 cores, host-permuted inputs,
x pre-transposed feature-major bf16):
    MM-A  hidden.T = (B*wnorm) @ xp.T          k=3840, out 512  bf16
    MM-B  yp[:,:3840] = hid @ A.T + xc @ s1.T  k=768, out 3840  bf16+fp8DR
    MM-C  y2 = (s2p*64) @ xp.T (feature-major) k=4096, out 256  fp8DR

v6 vs v5 (153.5us measured):
  - head: first w1/x chunks shrunk to 2 k-tiles and spread over FOUR
    dma queues (sync/scalar/vector/gpsimd).  v5's first x chunk (512KB
    on one queue) landed at 13.6us; the 2kt pieces land ~9.6us, so the
    real stream starts ~4us earlier.  Warm-ups trimmed 26 -> 20 to
    hand off seamlessly at ~9.6us (HAM ramp needs ~5us of continuous
    PE activity; an idle gap resets it).
  - tail: last B-row writes (yoa/yob of blk1 mt3) each split in half
    across scalar+sync, and the final MM-C mh=1 computed as two
    256-token halves so the kernel's last DMA is 64KB instead of
    480KB.  Teardown (fixed ~4us of semaphore cleanup) starts ~1.3us
    earlier.
"""

import numpy as np
import ml_dtypes

import concourse.bacc as bacc
import concourse.tile as tile
import concourse.mybir as mybir
from concourse.bass_utils import run_bass_kernel_spmd

N_CORES = 8
TOK = 8192
TPC = TOK // N_CORES  # 1024 tokens per core
N = 4096
RANK = 512
NKEEP = 3840
NCOMP = 256
BLK = 512             # token block (matmul moving N)
TT = 128              # token tile (stationary partition dim)
NBLK = TPC // BLK     # 2
KT_ALL = N // 128     # 32
KT_A = NKEEP // 128   # 30
KT_B = RANK // 128    # 4
NCH = 8
CW = NKEEP // NCH     # 480
XCK = 8               # k-tiles per full x chunk
S1S = 8.0
S2S = 64.0
NDUMMY = 20
HW = NKEEP // 2       # 1920, y half-row width
QW = NKEEP // 4       # 960, y quarter-row width

_BF16 = mybir.dt.bfloat16
_F32 = mybir.dt.float32
_F8 = mybir.dt.float8e4
_DR = mybir.MatmulPerfMode.DoubleRow


def _build_nc():
    nc = bacc.Bacc(None)
    x_d = nc.dram_tensor("x", [NBLK, 4, 128, XCK, BLK], _BF16, kind="ExternalInput")
    w1_d = nc.dram_tensor("w1", [2, 128, 15, RANK], _BF16, kind="ExternalInput")
    w2_d = nc.dram_tensor("w2", [128, KT_B, NKEEP], _BF16, kind="ExternalInput")
    s1_d = nc.dram_tensor("s1", [128, 2, NKEEP], _F8, kind="ExternalInput")
    s2_d = nc.dram_tensor("s2", [128, KT_ALL, NCOMP], _F8, kind="ExternalInput")
    y_d = nc.dram_tensor("y", [TPC, NKEEP], _BF16, kind="ExternalOutput")
    y2_d = nc.dram_tensor("y2", [NCOMP, TPC], _BF16, kind="ExternalOutput")

    with tile.TileContext(nc) as tc:
        with (
            tc.tile_pool(name="w1h", bufs=2) as w1h_pool,
            tc.tile_pool(name="w1q", bufs=1) as w1q_pool,
            tc.tile_pool(name="w1", bufs=3) as w1_pool,
            tc.tile_pool(name="w2", bufs=4) as w2_pool,
            tc.tile_pool(name="s1", bufs=1) as s1_pool,
            tc.tile_pool(name="s2", bufs=1) as s2_pool,
            tc.tile_pool(name="xh", bufs=2) as xh_pool,
            tc.tile_pool(name="xq", bufs=1) as xq_pool,
            tc.tile_pool(name="xt", bufs=7) as xt_pool,
            tc.tile_pool(name="x8", bufs=1) as x8_pool,
            tc.tile_pool(name="xc8", bufs=2) as xc8_pool,
            tc.tile_pool(name="u3", bufs=2) as u3_pool,
            tc.tile_pool(name="yoa", bufs=2) as yoa_pool,
            tc.tile_pool(name="yob", bufs=2) as yob_pool,
            tc.tile_pool(name="yc", bufs=2) as yc_pool,
            tc.tile_pool(name="wrm", bufs=1) as wrm_pool,
            tc.tile_pool(name="psA", bufs=4, space="PSUM") as psA,
            tc.tile_pool(name="psB", bufs=2, space="PSUM") as psB,
            tc.tile_pool(name="psC", bufs=2, space="PSUM") as psC,
        ):
            # --- tiles ---------------------------------------------------
            # blk0 w1 chunk layout: 2,2,4 then 8,8,6 k-tiles
            w1h = [w1h_pool.tile([128, 2, RANK], _BF16, name="w1ht")
                   for _ in range(2)]
            w1q = w1q_pool.tile([128, 4, RANK], _BF16)
            w1f = [w1_pool.tile([128, XCK, RANK], _BF16, name="w1sb")
                   for _ in range(3)]
            # (tile, kt_start, nkt) per A-chunk
            w1_chunks = [(w1h[0], 0, 2), (w1h[1], 2, 2), (w1q, 4, 4),
                         (w1f[0], 8, 8), (w1f[1], 16, 8), (w1f[2], 24, 6)]
            w2_sb = [w2_pool.tile([128, KT_B, 2 * CW], _BF16, name="w2sb")
                     for c in range(4)]
            s1_sb = s1_pool.tile([128, 2, NKEEP], _F8)
            s2_sb = s2_pool.tile([128, KT_ALL, NCOMP], _F8)
            xt0h = [xh_pool.tile([128, 2, BLK], _BF16, name="xht")
                    for _ in range(2)]
            xt0q = xq_pool.tile([128, 4, BLK], _BF16)
            xt_f = [xt_pool.tile([128, XCK, BLK], _BF16, name="xts")
                    for _ in range(7)]
            # per-block x chunk lists: (tile, kt_start, nkt)
            x_chunks = [
                [(xt0h[0], 0, 2), (xt0h[1], 2, 2), (xt0q, 4, 4),
                 (xt_f[0], 8, 8), (xt_f[1], 16, 8), (xt_f[2], 24, 8)],
                [(xt_f[3], 0, 8), (xt_f[4], 8, 8), (xt_f[5], 16, 8),
                 (xt_f[6], 24, 8)],
            ]
            x8_sb = x8_pool.tile([128, KT_ALL, BLK], _F8)

            # --- warm-up (HAM 8/8 before real MMs) -----------------------
            wrm = wrm_pool.tile([128, 128], _BF16)
            wps = psA.tile([128, 128], _F32, name="psa")
            nc.gpsimd.memset(wrm[:], 0.0)
            for i in range(NDUMMY):
                nc.tensor.matmul(wps[:], wrm[:], wrm[:], start=True, stop=True)

            # --- need-ordered loads, 3 queues (sync/scalar/gpsimd) -------
            nc.sync.dma_start(w1h[0][:], w1_d[0, :, :2])
            nc.scalar.dma_start(xt0h[0][:], x_d[0, 0, :, :2, :])
            nc.gpsimd.dma_start(w1h[1][:], w1_d[0, :, 2:4])
            nc.sync.dma_start(xt0h[1][:], x_d[0, 0, :, 2:4, :])
            nc.scalar.dma_start(w1q[:], w1_d[0, :, 4:8])
            nc.gpsimd.dma_start(xt0q[:], x_d[0, 0, :, 4:, :])
            nc.sync.dma_start(w1f[0][:, :7, :], w1_d[0, :, 8:])
            nc.scalar.dma_start(xt_f[0][:], x_d[0, 1])
            nc.gpsimd.dma_start(w1f[0][:, 7:8, :], w1_d[1, :, :1])
            nc.gpsimd.dma_start(w1f[1][:], w1_d[1, :, 1:9])
            nc.sync.dma_start(xt_f[1][:], x_d[0, 2])
            nc.scalar.dma_start(w1f[2][:, :6, :], w1_d[1, :, 9:])
            nc.gpsimd.dma_start(xt_f[2][:], x_d[0, 3])
            nc.sync.dma_start(s2_sb[:], s2_d[:])
            nc.scalar.dma_start(s1_sb[:], s1_d[:])
            nc.gpsimd.dma_start(w2_sb[0][:], w2_d[:, :, 0:2 * CW])
            nc.sync.dma_start(w2_sb[1][:], w2_d[:, :, 2 * CW:4 * CW])
            nc.scalar.dma_start(w2_sb[2][:], w2_d[:, :, 4 * CW:6 * CW])
            nc.gpsimd.dma_start(w2_sb[3][:], w2_d[:, :, 6 * CW:])
            nc.sync.dma_start(xt_f[3][:], x_d[1, 0])
            nc.scalar.dma_start(xt_f[4][:], x_d[1, 1])
            nc.gpsimd.dma_start(xt_f[5][:], x_d[1, 2])
            nc.sync.dma_start(xt_f[6][:], x_d[1, 3])

            # --- per-block compute ---------------------------------------
            def mm_a(blk):
                u3 = u3_pool.tile([128, KT_B, BLK], _BF16)
                psa = [psA.tile([128, BLK], _F32, name="psa")
                       for m in range(RANK // 128)]
                for (xt, kt0, nkt) in x_chunks[blk]:
                    for j in range(nkt):
                        kt = kt0 + j
                        if kt >= KT_A:
                            continue
                        ci = (0 if kt < 2 else 1 if kt < 4 else 2 if kt < 8
                              else 3 + (kt - 8) // 8)
                        wt, wkt0, _ = w1_chunks[ci]
                        for m in range(RANK // 128):
                            nc.tensor.matmul(
                                psa[m][:],
                                wt[:, kt - wkt0, m * 128:(m + 1) * 128],
                                xt[:, j, :],
                                start=(kt == 0),
                                stop=(kt == KT_A - 1),
                            )
                    # fp8 cast for MM-C, hidden in MM-A's DMA-paced window
                    nc.scalar.copy(out=x8_sb[:, kt0:kt0 + nkt, :], in_=xt[:])
                for m in range(RANK // 128):
                    nc.vector.tensor_copy(out=u3[:, m, :], in_=psa[m][:])
                xc8 = xc8_pool.tile([128, 2, BLK], _F8)
                nc.scalar.mul(xc8[:], x8_sb[:, 30:32, :], 1.0 / S1S)
                return u3, xc8

            def mm_b(blk, u3, xc8, last=False):
                t0 = blk * BLK
                for mt in range(BLK // TT):
                    yoa = yoa_pool.tile([128, HW], _BF16)
                    yob = yob_pool.tile([128, HW], _BF16)
                    fin = last and mt == BLK // TT - 1
                    r0, r1 = t0 + mt * TT, t0 + (mt + 1) * TT
                    for n in range(NCH):
                        ps = psB.tile([128, CW], _F32)
                        for kt in range(KT_B):
                            nc.tensor.matmul(
                                ps[:],
                                u3[:, kt, mt * TT:(mt + 1) * TT],
                                w2_sb[n // 2][:, kt,
                                              (n % 2) * CW:(n % 2 + 1) * CW],
                                start=(kt == 0),
                                stop=False,
                            )
                        nc.tensor.matmul(
                            ps[:],
                            xc8[:, :, mt * TT:(mt + 1) * TT],
                            s1_sb[:, :, n * CW:(n + 1) * CW],
                            start=False,
                            stop=True,
                            perf_mode=_DR,
                        )
                        dst = yoa if n < 4 else yob
                        nc.vector.tensor_copy(
                            out=dst[:, (n % 4) * CW:(n % 4 + 1) * CW], in_=ps[:]
                        )
                        if n == 3:
                            if fin:
                                nc.scalar.dma_start(y_d[r0:r1, :QW],
                                                    yoa[:, :QW])
                                nc.sync.dma_start(y_d[r0:r1, QW:HW],
                                                  yoa[:, QW:])
                            else:
                                nc.scalar.dma_start(y_d[r0:r1, :HW], yoa[:])
                    if fin:
                        nc.scalar.dma_start(y_d[r0:r1, HW:HW + QW],
                                            yob[:, :QW])
                        nc.sync.dma_start(y_d[r0:r1, HW + QW:], yob[:, QW:])
                    else:
                        nc.scalar.dma_start(y_d[r0:r1, HW:], yob[:])

            def mm_c(blk, split=False):
                t0 = blk * BLK
                for mh in range(NCOMP // 128):
                    m0, m1 = mh * 128, (mh + 1) * 128
                    if split and mh == 1:
                        # two 256-token halves so the kernel's last DMA
                        # is 64KB and issues right after its matmuls
                        for h in range(2):
                            c0 = h * 256
                            ps = psC.tile([128, BLK], _F32)
                            for k2 in range(KT_ALL // 2):
                                nc.tensor.matmul(
                                    ps[:, :256],
                                    s2_sb[:, 2 * k2:2 * k2 + 2, m0:m1],
                                    x8_sb[:, 2 * k2:2 * k2 + 2, c0:c0 + 256],
                                    start=(k2 == 0),
                                    stop=(k2 == KT_ALL // 2 - 1),
                                    perf_mode=_DR,
                                )
                            yc = yc_pool.tile([128, 256], _BF16)
                            nc.scalar.mul(yc[:], ps[:, :256], 1.0 / S2S)
                            eng = nc.gpsimd if h == 0 else nc.sync
                            eng.dma_start(
                                y2_d[m0:m1, t0 + c0:t0 + c0 + 256], yc[:]
                            )
                        continue
                    ps = psC.tile([128, BLK], _F32)
                    for k2 in range(KT_ALL // 2):
                        nc.tensor.matmul(
                            ps[:],
                            s2_sb[:, 2 * k2:2 * k2 + 2, m0:m1],
                            x8_sb[:, 2 * k2:2 * k2 + 2, :],
                            start=(k2 == 0),
                            stop=(k2 == KT_ALL // 2 - 1),
                            perf_mode=_DR,
                        )
                    yc = yc_pool.tile([128, BLK], _BF16)
                    nc.scalar.mul(yc[:], ps[:], 1.0 / S2S)
                    nc.gpsimd.dma_start(y2_d[m0:m1, t0:t0 + BLK], yc[:])

            # blk0: A,C,B -- C bridges the w2-arrival wait, keeps HAM warm.
            # blk1: A,B,C -- the tiny final y2 half-write ends the kernel.
            u3, xc8 = mm_a(0)
            mm_c(0)
            mm_b(0, u3, xc8)
            u3, xc8 = mm_a(1)
            mm_b(1, u3, xc8, last=True)
            mm_c(1, split=True)
    nc.finalize()
    return nc


_NC_CACHE = {}


def get_nc():
    if "nc" not in _NC_CACHE:
        _NC_CACHE["nc"] = _build_nc()
    return _NC_CACHE["nc"]


def _prep(A, B, sparse_weights1, sparse_weights2, weights_norms_rowwise,
          col_idx, col_comp_idx):
    bf16 = ml_dtypes.bfloat16
    f8 = ml_dtypes.float8_e4m3
    perm_in = np.concatenate([col_idx, col_comp_idx])
    w1t = (B * weights_norms_rowwise[None, :]).T.astype(np.float32)
    w1 = np.ascontiguousarray(
        w1t.reshape(2, 15, 128, RANK).transpose(0, 2, 1, 3)
    ).astype(bf16)
    w2 = np.ascontiguousarray(
        A.T.astype(np.float32).reshape(KT_B, 128, NKEEP).transpose(1, 0, 2)
    ).astype(bf16)
    s1 = np.ascontiguousarray(
        (sparse_weights1.T * S1S).astype(np.float32)
        .reshape(2, 128, NKEEP).transpose(1, 0, 2)
    ).astype(f8)
    s2t = (sparse_weights2[:, perm_in].T * S2S).astype(np.float32)
    s2 = np.ascontiguousarray(
        s2t.reshape(KT_ALL, 128, NCOMP).transpose(1, 0, 2)
    ).astype(f8)
    return w1, w2, s1, s2, perm_in


def kernel(x, A, B, sparse_weights1, sparse_weights2, weights_norms_rowwise,
           col_idx, col_comp_idx, row_idx, row_comp_idx):
    bf16 = ml_dtypes.bfloat16
    x = np.asarray(x, dtype=np.float32)
    w1, w2, s1, s2, perm_in = _prep(
        np.asarray(A, np.float32), np.asarray(B, np.float32),
        np.asarray(sparse_weights1, np.float32),
        np.asarray(sparse_weights2, np.float32),
        np.asarray(weights_norms_rowwise, np.float32),
        np.asarray(col_idx), np.asarray(col_comp_idx),
    )
    row_idx = np.asarray(row_idx)
    row_comp_idx = np.asarray(row_comp_idx)

    xs = x.reshape(TOK, N)
    in_maps = []
    for c in range(N_CORES):
        xcT = xs[c * TPC:(c + 1) * TPC][:, perm_in].T
        xb = np.ascontiguousarray(
            xcT.reshape(4, XCK, 128, NBLK, BLK).transpose(3, 0, 2, 1, 4)
        ).astype(bf16)
        in_maps.append({"x": xb, "w1": w1, "w2": w2, "s1": s1, "s2": s2})

    nc = get_nc()
    res = run_bass_kernel_spmd(nc, in_maps, core_ids=list(range(N_CORES)))
    globals()["_LAST_RESULTS"] = res
    y_rows = np.concatenate(
        [np.asarray(res.results[c]["y"], dtype=np.float32) for c in range(N_CORES)],
        axis=0,
    )
    y_comp = np.concatenate(
        [np.asarray(res.results[c]["y2"], dtype=np.float32) for c in range(N_CORES)],
        axis=1,
    )
    y = np.empty((TOK, N), dtype=np.float32)
    y[:, row_idx] = y_rows
    y[:, row_comp_idx] = y_comp.T
    return np.ascontiguousarray(y.reshape(x.shape))


# revision 5
# speedup vs baseline: 1.1222x; 1.1222x over previous
"""Low_Rank_linear Trainium2 kernel, v6b.

Per 512-token block (data-parallel over 8 cores, host-permuted inputs,
x pre-transposed feature-major bf16):
    MM-A  hidden.T = (B*wnorm) @ xp.T          k=3840, out 512  bf16
    MM-B  yp[:,:3840] = hid @ A.T + xc @ s1.T  k=768, out 3840  bf16+fp8DR
    MM-C  y2 = (s2p*64) @ xp.T (feature-major) k=4096, out 256  fp8DR

v6b vs v5 (153.5us measured):
  - only TWO hardware DMA queues exist (sync ~145GB/s, scalar
    ~145GB/s); gpsimd is a ~42GB/s software queue.  gpsimd carries
    only late-needed bytes: s2 (two halves) + one blk1 x chunk.
  - head: first w1/x chunks shrunk to 2 k-tiles, then 4-kt chunks
    through kt29, w1/x of the same kt range on opposite HW queues.
    First real matmul ~9.6us (v5: 13.6us).  Mid-A0 supply shortfall
    (~300 needed vs ~290GB/s) shows up as micro-stalls on 4-kt
    boundaries, which do NOT trip the HAM 4/8 re-throttle (v6's
    multi-us gaps did).
  - MM-B restructured into four n-quarters (quarter q uses only
    w2_sb[q]), so w2 chunk deadlines spread over ~26us instead of
    all inside the first row-tile.  y written as [128,960] quarter
    chunks right after each row-tile finishes a quarter.
  - tail: final MM-C mh=1 computed as two 256-token halves so the
    kernel's last DMA is 64KB.
"""

import numpy as np
import ml_dtypes

import concourse.bacc as bacc
import concourse.tile as tile
import concourse.mybir as mybir
from concourse.bass_utils import run_bass_kernel_spmd

N_CORES = 8
TOK = 8192
TPC = TOK // N_CORES  # 1024 tokens per core
N = 4096
RANK = 512
NKEEP = 3840
NCOMP = 256
BLK = 512             # token block (matmul moving N)
TT = 128              # token tile (stationary partition dim)
NBLK = TPC // BLK     # 2
KT_ALL = N // 128     # 32
KT_A = NKEEP // 128   # 30
KT_B = RANK // 128    # 4
NCH = 8
CW = NKEEP // NCH     # 480
XCK = 8               # k-tiles per full x chunk
S1S = 8.0
S2S = 64.0
NDUMMY = 20
QW = 2 * CW           # 960, y quarter-row width

_BF16 = mybir.dt.bfloat16
_F32 = mybir.dt.float32
_F8 = mybir.dt.float8e4
_DR = mybir.MatmulPerfMode.DoubleRow


def _build_nc():
    nc = bacc.Bacc(None)
    x_d = nc.dram_tensor("x", [NBLK, 4, 128, XCK, BLK], _BF16, kind="ExternalInput")
    w1_d = nc.dram_tensor("w1", [2, 128, 15, RANK], _BF16, kind="ExternalInput")
    w2_d = nc.dram_tensor("w2", [128, KT_B, NKEEP], _BF16, kind="ExternalInput")
    s1_d = nc.dram_tensor("s1", [128, 2, NKEEP], _F8, kind="ExternalInput")
    s2_d = nc.dram_tensor("s2", [128, KT_ALL, NCOMP], _F8, kind="ExternalInput")
    y_d = nc.dram_tensor("y", [TPC, NKEEP], _BF16, kind="ExternalOutput")
    y2_d = nc.dram_tensor("y2", [NCOMP, TPC], _BF16, kind="ExternalOutput")

    with tile.TileContext(nc) as tc:
        with (
            tc.tile_pool(name="w1h", bufs=2) as w1h_pool,
            tc.tile_pool(name="w1q", bufs=6) as w1q_pool,
            tc.tile_pool(name="w1t", bufs=1) as w1t_pool,
            tc.tile_pool(name="w2", bufs=4) as w2_pool,
            tc.tile_pool(name="s1", bufs=1) as s1_pool,
            tc.tile_pool(name="s2", bufs=1) as s2_pool,
            tc.tile_pool(name="xh", bufs=2) as xh_pool,
            tc.tile_pool(name="xq", bufs=7) as xq_pool,
            tc.tile_pool(name="xt", bufs=4) as xt_pool,
            tc.tile_pool(name="x8", bufs=1) as x8_pool,
            tc.tile_pool(name="xc8", bufs=2) as xc8_pool,
            tc.tile_pool(name="u3", bufs=2) as u3_pool,
            tc.tile_pool(name="yq", bufs=4) as yq_pool,
            tc.tile_pool(name="yc", bufs=2) as yc_pool,
            tc.tile_pool(name="wrm", bufs=1) as wrm_pool,
            tc.tile_pool(name="psA", bufs=4, space="PSUM") as psA,
            tc.tile_pool(name="psB", bufs=2, space="PSUM") as psB,
            tc.tile_pool(name="psC", bufs=2, space="PSUM") as psC,
        ):
            # --- tiles ---------------------------------------------------
            w1h = [w1h_pool.tile([128, 2, RANK], _BF16, name="w1ht")
                   for _ in range(2)]
            w1q = [w1q_pool.tile([128, 4, RANK], _BF16, name="w1qt")
                   for _ in range(6)]
            w1t = w1t_pool.tile([128, 2, RANK], _BF16)
            # (tile, kt_start, nkt) per A-chunk
            w1_chunks = ([(w1h[0], 0, 2), (w1h[1], 2, 2)]
                         + [(w1q[i], 4 + 4 * i, 4) for i in range(6)]
                         + [(w1t, 28, 2)])
            w2_sb = [w2_pool.tile([128, KT_B, QW], _BF16, name="w2sb")
                     for c in range(4)]
            s1_sb = s1_pool.tile([128, 2, NKEEP], _F8)
            s2_sb = s2_pool.tile([128, KT_ALL, NCOMP], _F8)
            xh = [xh_pool.tile([128, 2, BLK], _BF16, name="xht")
                  for _ in range(2)]
            xq = [xq_pool.tile([128, 4, BLK], _BF16, name="xqt")
                  for _ in range(7)]
            xt_f = [xt_pool.tile([128, XCK, BLK], _BF16, name="xts")
                    for _ in range(4)]
            # per-block x chunk lists: (tile, kt_start, nkt)
            x_chunks = [
                [(xh[0], 0, 2), (xh[1], 2, 2)]
                + [(xq[i], 4 + 4 * i, 4) for i in range(7)],
                [(xt_f[i], 8 * i, 8) for i in range(4)],
            ]
            x8_sb = x8_pool.tile([128, KT_ALL, BLK], _F8)

            # --- warm-up (HAM 8/8 before real MMs) -----------------------
            wrm = wrm_pool.tile([128, 128], _BF16)
            wps = psA.tile([128, 128], _F32, name="psa")
            nc.gpsimd.memset(wrm[:], 0.0)
            for i in range(NDUMMY):
                nc.tensor.matmul(wps[:], wrm[:], wrm[:], start=True, stop=True)

            # --- need-ordered loads ---------------------------------------
            # two HW queues (sync, scalar): w1/x of the same kt range on
            # opposite queues; scalar's load list must END by ~40us so the
            # y quarter-writes it carries never queue behind loads.
            nc.sync.dma_start(w1h[0][:], w1_d[0, :, 0:2])
            nc.scalar.dma_start(xh[0][:], x_d[0, 0, :, 0:2, :])
            nc.sync.dma_start(xh[1][:], x_d[0, 0, :, 2:4, :])
            nc.scalar.dma_start(w1h[1][:], w1_d[0, :, 2:4])
            nc.sync.dma_start(xq[0][:], x_d[0, 0, :, 4:8, :])
            nc.scalar.dma_start(w1q[0][:], w1_d[0, :, 4:8])
            nc.sync.dma_start(w1q[1][:], w1_d[0, :, 8:12])
            nc.scalar.dma_start(xq[1][:], x_d[0, 1, :, 0:4, :])
            nc.sync.dma_start(xq[2][:], x_d[0, 1, :, 4:8, :])
            nc.scalar.dma_start(w1q[2][:, 0:3, :], w1_d[0, :, 12:15])
            nc.scalar.dma_start(w1q[2][:, 3:4, :], w1_d[1, :, 0:1])
            nc.sync.dma_start(w1q[3][:], w1_d[1, :, 1:5])
            nc.scalar.dma_start(xq[3][:], x_d[0, 2, :, 0:4, :])
            nc.sync.dma_start(xq[4][:], x_d[0, 2, :, 4:8, :])
            nc.scalar.dma_start(w1q[4][:], w1_d[1, :, 5:9])
            nc.sync.dma_start(w1q[5][:], w1_d[1, :, 9:13])
            nc.scalar.dma_start(xq[5][:], x_d[0, 3, :, 0:4, :])
            nc.sync.dma_start(xq[6][:], x_d[0, 3, :, 4:8, :])
            nc.scalar.dma_start(w1t[:], w1_d[1, :, 13:15])
            nc.scalar.dma_start(s1_sb[:], s1_d[:])          # scalar ENDS here
            nc.sync.dma_start(w2_sb[0][:], w2_d[:, :, 0:QW])
            nc.sync.dma_start(w2_sb[1][:], w2_d[:, :, QW:2 * QW])
            nc.sync.dma_start(w2_sb[2][:], w2_d[:, :, 2 * QW:3 * QW])
            nc.sync.dma_start(w2_sb[3][:], w2_d[:, :, 3 * QW:])
            nc.sync.dma_start(xt_f[1][:], x_d[1, 1])
            nc.sync.dma_start(xt_f[2][:], x_d[1, 2])
            nc.sync.dma_start(xt_f[3][:], x_d[1, 3])
            # slow software queue: late-needed bytes only
            nc.gpsimd.dma_start(s2_sb[:, 0:16, :], s2_d[:, 0:16, :])
            nc.gpsimd.dma_start(s2_sb[:, 16:32, :], s2_d[:, 16:32, :])
            nc.gpsimd.dma_start(xt_f[0][:], x_d[1, 0])

            # --- per-block compute ---------------------------------------
            def mm_a(blk):
                u3 = u3_pool.tile([128, KT_B, BLK], _BF16)
                psa = [psA.tile([128, BLK], _F32, name="psa")
                       for m in range(RANK // 128)]
                for (xt, kt0, nkt) in x_chunks[blk]:
                    for j in range(nkt):
                        kt = kt0 + j
                        if kt >= KT_A:
                            continue
                        ci = (kt // 2 if kt < 4 else
                              2 + (kt - 4) // 4 if kt < 28 else 8)
                        wt, wkt0, _ = w1_chunks[ci]
                        for m in range(RANK // 128):
                            nc.tensor.matmul(
                                psa[m][:],
                                wt[:, kt - wkt0, m * 128:(m + 1) * 128],
                                xt[:, j, :],
                                start=(kt == 0),
                                stop=(kt == KT_A - 1),
                            )
                    # fp8 cast for MM-C, hidden in MM-A's DMA-paced window
                    nc.scalar.copy(out=x8_sb[:, kt0:kt0 + nkt, :], in_=xt[:])
                for m in range(RANK // 128):
                    nc.vector.tensor_copy(out=u3[:, m, :], in_=psa[m][:])
                xc8 = xc8_pool.tile([128, 2, BLK], _F8)
                nc.scalar.mul(xc8[:], x8_sb[:, 30:32, :], 1.0 / S1S)
                return u3, xc8

            def mm_b(blk, u3, xc8):
                t0 = blk * BLK
                for q in range(4):          # n-quarter: uses only w2_sb[q]
                    for mt in range(BLK // TT):
                        yq = yq_pool.tile([128, QW], _BF16)
                        r0, r1 = t0 + mt * TT, t0 + (mt + 1) * TT
                        for h in range(2):
                            n = 2 * q + h
                            ps = psB.tile([128, CW], _F32)
                            for kt in range(KT_B):
                                nc.tensor.matmul(
                                    ps[:],
                                    u3[:, kt, mt * TT:(mt + 1) * TT],
                                    w2_sb[q][:, kt, h * CW:(h + 1) * CW],
                                    start=(kt == 0),
                                    stop=False,
                                )
                            nc.tensor.matmul(
                                ps[:],
                                xc8[:, :, mt * TT:(mt + 1) * TT],
                                s1_sb[:, :, n * CW:(n + 1) * CW],
                                start=False,
                                stop=True,
                                perf_mode=_DR,
                            )
                            nc.vector.tensor_copy(
                                out=yq[:, h * CW:(h + 1) * CW], in_=ps[:]
                            )
                        nc.scalar.dma_start(
                            y_d[r0:r1, q * QW:(q + 1) * QW], yq[:]
                        )

            def mm_c(blk, split=False):
                t0 = blk * BLK
                for mh in range(NCOMP // 128):
                    m0, m1 = mh * 128, (mh + 1) * 128
                    if split and mh == 1:
                        # two 256-token halves so the kernel's last DMA
                        # is 64KB and issues right after its matmuls
                        for h in range(2):
                            c0 = h * 256
                            ps = psC.tile([128, BLK], _F32)
                            for k2 in range(KT_ALL // 2):
                                nc.tensor.matmul(
                                    ps[:, :256],
                                    s2_sb[:, 2 * k2:2 * k2 + 2, m0:m1],
                                    x8_sb[:, 2 * k2:2 * k2 + 2, c0:c0 + 256],
                                    start=(k2 == 0),
                                    stop=(k2 == KT_ALL // 2 - 1),
                                    perf_mode=_DR,
                                )
                            yc = yc_pool.tile([128, 256], _BF16)
                            nc.scalar.mul(yc[:], ps[:, :256], 1.0 / S2S)
                            eng = nc.gpsimd if h == 0 else nc.sync
                            eng.dma_start(
                                y2_d[m0:m1, t0 + c0:t0 + c0 + 256], yc[:]
                            )
                        continue
                    ps = psC.tile([128, BLK], _F32)
                    for k2 in range(KT_ALL // 2):
                        nc.tensor.matmul(
                            ps[:],
                            s2_sb[:, 2 * k2:2 * k2 + 2, m0:m1],
                            x8_sb[:, 2 * k2:2 * k2 + 2, :],
                            start=(k2 == 0),
                            stop=(k2 == KT_ALL // 2 - 1),
                            perf_mode=_DR,
                        )
                    yc = yc_pool.tile([128, BLK], _BF16)
                    nc.scalar.mul(yc[:], ps[:], 1.0 / S2S)
                    nc.gpsimd.dma_start(y2_d[m0:m1, t0:t0 + BLK], yc[:])

            # blk0: A,C,B -- C bridges the w2-arrival wait, keeps HAM warm.
            # blk1: A,B,C -- the tiny final y2 half-write ends the kernel.
            u3, xc8 = mm_a(0)
            mm_c(0)
            mm_b(0, u3, xc8)
            u3, xc8 = mm_a(1)
            mm_b(1, u3, xc8)
            mm_c(1, split=True)
    nc.finalize()
    return nc


_NC_CACHE = {}


def get_nc():
    if "nc" not in _NC_CACHE:
        _NC_CACHE["nc"] = _build_nc()
    return _NC_CACHE["nc"]


def _prep(A, B, sparse_weights1, sparse_weights2, weights_norms_rowwise,
          col_idx, col_comp_idx):
    bf16 = ml_dtypes.bfloat16
    f8 = ml_dtypes.float8_e4m3
    perm_in = np.concatenate([col_idx, col_comp_idx])
    w1t = (B * weights_norms_rowwise[None, :]).T.astype(np.float32)
    w1 = np.ascontiguousarray(
        w1t.reshape(2, 15, 128, RANK).transpose(0, 2, 1, 3)
    ).astype(bf16)
    w2 = np.ascontiguousarray(
        A.T.astype(np.float32).reshape(KT_B, 128, NKEEP).transpose(1, 0, 2)
    ).astype(bf16)
    s1 = np.ascontiguousarray(
        (sparse_weights1.T * S1S).astype(np.float32)
        .reshape(2, 128, NKEEP).transpose(1, 0, 2)
    ).astype(f8)
    s2t = (sparse_weights2[:, perm_in].T * S2S).astype(np.float32)
    s2 = np.ascontiguousarray(
        s2t.reshape(KT_ALL, 128, NCOMP).transpose(1, 0, 2)
    ).astype(f8)
    return w1, w2, s1, s2, perm_in


def kernel(x, A, B, sparse_weights1, sparse_weights2, weights_norms_rowwise,
           col_idx, col_comp_idx, row_idx, row_comp_idx):
    bf16 = ml_dtypes.bfloat16
    x = np.asarray(x, dtype=np.float32)
    w1, w2, s1, s2, perm_in = _prep(
        np.asarray(A, np.float32), np.asarray(B, np.float32),
        np.asarray(sparse_weights1, np.float32),
        np.asarray(sparse_weights2, np.float32),
        np.asarray(weights_norms_rowwise, np.float32),
        np.asarray(col_idx), np.asarray(col_comp_idx),
    )
    row_idx = np.asarray(row_idx)
    row_comp_idx = np.asarray(row_comp_idx)

    xs = x.reshape(TOK, N)
    in_maps = []
    for c in range(N_CORES):
        xcT = xs[c * TPC:(c + 1) * TPC][:, perm_in].T
        xb = np.ascontiguousarray(
            xcT.reshape(4, XCK, 128, NBLK, BLK).transpose(3, 0, 2, 1, 4)
        ).astype(bf16)
        in_maps.append({"x": xb, "w1": w1, "w2": w2, "s1": s1, "s2": s2})

    nc = get_nc()
    res = run_bass_kernel_spmd(nc, in_maps, core_ids=list(range(N_CORES)))
    globals()["_LAST_RESULTS"] = res
    y_rows = np.concatenate(
        [np.asarray(res.results[c]["y"], dtype=np.float32) for c in range(N_CORES)],
        axis=0,
    )
    y_comp = np.concatenate(
        [np.asarray(res.results[c]["y2"], dtype=np.float32) for c in range(N_CORES)],
        axis=1,
    )
    y = np.empty((TOK, N), dtype=np.float32)
    y[:, row_idx] = y_rows
    y[:, row_comp_idx] = y_comp.T
    return np.ascontiguousarray(y.reshape(x.shape))
